# revision 17
# baseline (speedup 1.0000x reference)
"""Multi-head attention (16 heads, d_model=1024, B=2, S=2048) on 8 TRN2 cores.

Sharding: core c -> batch b = c//4, query slab s = c%4 (512 rows).
Each core projects K/V for its whole batch (replicated over the 4 slab
cores of that batch), projects its Q slab, computes transposed scores
per head (scores_T[sk,sq] = K_h Q_h^T), exp without max-subtraction
(scores are O(5)), gets softmax denominators from a ones-column appended
to V during the PV matmul, normalizes, and runs the output projection
for its slab (no cross-core reduction needed). attn is produced
transposed per (b,h) and transposed back on host during unshard.

Raw Bass with manual semaphores: this walrus build allows only ONE
sync-wait per DMA/matmul instruction, so multi-deps ride on standalone
wait_ge instructions. The softmax reciprocal row is replicated across
partitions with a K=1 ones-matmul on the PE. Normalize multiplies are
split DVE (sk-tiles 0-11) / GpSimd (12-15); all stores issue from SP.
Wq is prefetched into a scratch arena that later holds probs/nrm
(disjoint lifetimes), so projection phases overlap their weight loads.

Matmul dtypes: projections + output projection in float32r (fp32 with
11-bit mantissa at full PE rate; inputs pre-rounded on host), attention
operands (K^T/Q^T heads, V+ones, probs) in bf16, accumulation fp32.
"""
import numpy as np

B, S, DM, H, DH = 2, 2048, 1024, 16, 64
NCORES, SLABS = 8, 4
SQ = S // SLABS      # 512
P = 128
KT = DM // P         # 8
NT = S // P          # 16 key tiles

_cache = {}


def _round_f32r(a):
    a = np.ascontiguousarray(a, dtype=np.float32)
    u = a.view(np.uint32).astype(np.uint64)
    u = (u + 0x7FF + ((u >> 12) & 1)) & 0xFFFFF000
    return u.astype(np.uint32).view(np.float32)


def _build():
    import concourse.bass as bass
    import concourse.mybir as mybir

    f32, f32r, bf16 = mybir.dt.float32, mybir.dt.float32r, mybir.dt.bfloat16
    Exp = mybir.ActivationFunctionType.Exp

    nc = bass.Bass(trn_type="TRN2", target_bir_lowering=False)

    wqT = nc.dram_tensor("wqT", [DM, DM], f32r, kind="ExternalInput")
    wkT = nc.dram_tensor("wkT", [DM, DM], f32r, kind="ExternalInput")
    wvT = nc.dram_tensor("wvT", [DM, DM], f32r, kind="ExternalInput")
    woT = nc.dram_tensor("woT", [DM, DM], f32r, kind="ExternalInput")
    kTd = nc.dram_tensor("kT", [DM, S], f32r, kind="ExternalInput")
    vTd = nc.dram_tensor("vT", [DM, S], f32r, kind="ExternalInput")
    qTd = nc.dram_tensor("qT", [DM, SQ], f32r, kind="ExternalInput")
    onesd = nc.dram_tensor("ones", [1, P], f32r, kind="ExternalInput")
    attnT = nc.dram_tensor("attnT", [H, S, SQ], f32, kind="ExternalOutput")
    out_p = nc.dram_tensor("out_p", [SQ, DM], f32, kind="ExternalOutput")

    from contextlib import ExitStack
    ctx = ExitStack()
    sb, ps = nc.sbuf_tensor, nc.psum_tensor

    w_ar = ctx.enter_context(sb("w_ar", [P, KT, DM], f32r))          # 32K: wk, wv, wo
    in_ar = ctx.enter_context(sb("in_ar", [P, KT, SQ], f32r))        # 16K: kst/qst/vst
    khT = ctx.enter_context(sb("khT", [P, KT, S], bf16))             # 32K
    qhT = ctx.enter_context(sb("qhT", [P, KT, SQ], bf16))            # 8K
    vha = ctx.enter_context(sb("vha", [P, NT, H * (DH + 1)], bf16))  # 32.5K
    aoT = ctx.enter_context(sb("aoT", [P, KT, SQ], f32r))            # 16K
    scr = ctx.enter_context(sb("scr", [P, 8192], f32r))              # 32K: wq | nrm | osb
    probs = ctx.enter_context(sb("probs", [P, NT + 4, SQ], bf16))    # 20K ring of 20 tiles
    rbc_sb = ctx.enter_context(sb("rbc_sb", [P, SQ], f32))           # 2K
    rr = ctx.enter_context(sb("rr", [1, SQ], f32))
    rr_r = ctx.enter_context(sb("rr_r", [1, SQ], f32r))
    ones_r = ctx.enter_context(sb("ones_r", [1, P], f32r))

    acc_s = [ctx.enter_context(ps(f"acc_s{i}", [P, 512], f32)) for i in range(2)]
    acc_b = [ctx.enter_context(ps(f"acc_b{i}", [P, 1024], f32)) for i in range(2)]
    pvo = ctx.enter_context(ps("pvo", [P, SQ], f32))   # pvo buffers: [pvo, acc_s[0]]
    rbc_ps = ctx.enter_context(ps("rbc_ps", [P, SQ], f32))

    kst = [in_ar[:, :, :], in_ar[:, :, :]]          # single buffer
    vst = [in_ar[:, :, 128 * i:128 * (i + 1)] for i in range(2)]
    qst = aoT[:, :, :]       # aoT is idle until P4; qT loads here eagerly
    # scratch views (wq lifetime: P1..P2; nrm slots: P4; osb: P5) — all f32r
    wq = scr[:, :].rearrange("p (o f) -> p o f", f=DM)                      # [P,8,1024]
    nrm_slot = [scr[:, 2048 * i:2048 * (i + 1)]
                .rearrange("p (t f) -> p t f", f=SQ) for i in range(4)]     # [P,4,512] f32r
    osb = scr[:, 0:4096].rearrange("p (m f) -> p m f", f=DM)                # [P,4,1024]

    sems = ExitStack()
    dL = sems.enter_context(nc.semaphore("dL"))
    dS0 = sems.enter_context(nc.semaphore("dS0"))
    dS1 = sems.enter_context(nc.semaphore("dS1"))
    sPE = sems.enter_context(nc.semaphore("sPE"))
    sDVE = sems.enter_context(nc.semaphore("sDVE"))
    sACT = sems.enter_context(nc.semaphore("sACT"))
    sPOOL = sems.enter_context(nc.semaphore("sPOOL"))

    # milestones
    pe_p1 = lambda skb, mt: 8 * skb + mt + 1            # 1..32
    pe_p2 = lambda mt: 32 + mt + 1                      # 33..40
    pe_p3 = lambda st: 40 + st + 1                      # 41..56
    pe_qk = lambda h, g: 56 + 10 * h + g + 1
    pe_pvo = lambda h: 56 + 10 * h + 9
    pe_rbc = lambda h: 56 + 10 * h + 10                 # end P4: 216
    pe_p5 = lambda m: 216 + m + 1

    dv_rr = lambda h: 56 + 6 * h + 1
    dv_rbc = lambda h: 56 + 6 * h + 2
    dv_aot = lambda h: 56 + 6 * h + 3
    dv_nrm = lambda h, c: 56 + 6 * h + 4 + c            # c=0..2
    dv_p5 = lambda m: 152 + m + 1                       # 153..156

    ac_exp = lambda h, g: 8 * h + g + 1                 # 1..128
    po_nrm = lambda h: h + 1                            # 1..16
    pslot = lambda h, t: (16 * h + t) % 20              # probs ring slot

    # load order: 1 wk, 2 kst0, 3 kst1, 4 wq, 5 kst2, 6 kst3, 7 qst, 8 wv,
    #             9..24 vst, 25 wo
    with nc.Block() as block:

        @block.gpsimd
        def _(g):
            g.dma_start(w_ar[:], wkT.rearrange("(o p) f -> p o f", p=P)).then_inc(dL, 16)
            g.dma_start(kst[0], kTd[:, 0:SQ].rearrange("(o p) f -> p o f", p=P)).then_inc(dL, 16)
            g.dma_start(wq, wqT.rearrange("(o p) f -> p o f", p=P)).then_inc(dL, 16)
            g.wait_ge(sPE, 8)
            g.dma_start(kst[1], kTd[:, SQ:2 * SQ].rearrange("(o p) f -> p o f", p=P)).then_inc(dL, 16)
            g.dma_start(qst, qTd.rearrange("(o p) f -> p o f", p=P)).then_inc(dL, 16)
            for skb in (2, 3):
                g.wait_ge(sPE, 8 * skb)
                g.dma_start(kst[skb % 2], kTd[:, SQ * skb:SQ * (skb + 1)]
                            .rearrange("(o p) f -> p o f", p=P)).then_inc(dL, 16)
            g.wait_ge(sPE, 32)
            g.dma_start(w_ar[:], wvT.rearrange("(o p) f -> p o f", p=P)).then_inc(dL, 16)
            for st in range(NT):
                if st < 2:
                    g.wait_ge(sPE, 32)
                else:
                    g.wait_ge(sPE, pe_p3(st - 2))
                g.dma_start(vst[st % 2], vTd[:, P * st:P * (st + 1)]
                            .rearrange("(o p) f -> p o f", p=P)).then_inc(dL, 16)
            g.wait_ge(sPE, 56)
            g.dma_start(w_ar[:], woT.rearrange("(o p) f -> p o f", p=P)).then_inc(dL, 16)
            g.dma_start(ones_r[:, :], onesd[:, :]).then_inc(dL, 16)
            # P4: normalize sk-tiles 12..15 + own store stream
            for h in range(H):
                g.wait_ge(sDVE, dv_rbc(h))
                if h >= 1:
                    g.wait_ge(dS1, 16 * h)
                s0 = pslot(h, 12)
                for t2 in range(4):
                    mm = g.tensor_mul(nrm_slot[3][:, t2, :], probs[:, s0 + t2, :],
                                      rbc_sb[:, :])
                mm.then_inc(sPOOL, 1)
                g.dma_start(attnT[h, 512 * 3:512 * 4, :]
                            .rearrange("(o p) f -> p o f", p=P),
                            nrm_slot[3][:, :, :].bitcast(f32)).then_inc(dS1, 16)

        @block.tensor
        def _(t):
            # P1: khT
            dl_need = {0: 2, 1: 4, 2: 6, 3: 7}
            for skb in range(4):
                t.wait_ge(dL, 16 * dl_need[skb])
                for mt in range(KT):
                    idx = 8 * skb + mt
                    if idx >= 2:
                        t.wait_ge(sDVE, idx - 1)
                    for k in range(KT):
                        mm = t.matmul(acc_s[idx % 2][:, :],
                                      w_ar[:, k, P * mt:P * (mt + 1)],
                                      kst[skb % 2][:, k, :],
                                      start=(k == 0), stop=(k == KT - 1))
                    mm.then_inc(sPE, 1)
            # P2: qhT (weights from scr alias, activations from aoT alias)
            t.wait_ge(dL, 16 * 5)
            for mt in range(KT):
                idx = 32 + mt
                t.wait_ge(sDVE, idx - 1)
                for k in range(KT):
                    mm = t.matmul(acc_s[idx % 2][:, :],
                                  wq[:, k, P * mt:P * (mt + 1)],
                                  qst[:, k, :],
                                  start=(k == 0), stop=(k == KT - 1))
                mm.then_inc(sPE, 1)
            # P3: vh
            for st in range(NT):
                t.wait_ge(dL, 16 * (9 + st))
                if st >= 2:
                    t.wait_ge(sDVE, 41 + (st - 2))
                for half in range(2):
                    for k in range(KT):
                        mm = t.matmul(acc_b[st % 2][:, 512 * half:512 * (half + 1)],
                                      vst[st % 2][:, k, :],
                                      w_ar[:, k, 512 * half:512 * (half + 1)],
                                      start=(k == 0), stop=(k == KT - 1))
                mm.then_inc(sPE, 1)
            # P4
            t.wait_ge(sDVE, 56)

            def qk(h, gg):
                off, mth = (h % 2) * DH, h // 2
                if h == 0 and gg < 2:
                    t.wait_ge(sDVE, 55 + gg)
                else:
                    t.wait_ge(sACT, ac_exp(h, gg) - 2)
                for j in range(2):
                    ti = 2 * gg + j
                    mm = t.matmul(acc_b[gg % 2][:, 512 * j:512 * (j + 1)],
                                  khT[off:off + DH, mth, P * ti:P * (ti + 1)],
                                  qhT[off:off + DH, mth, :],
                                  start=True, stop=True)
                mm.then_inc(sPE, 1)

            def pv(h, gg):
                t.wait_ge(sACT, ac_exp(h, gg))
                if gg == 0 and h >= 2:
                    t.wait_ge(sDVE, dv_aot(h - 2))
                buf = pvo if h % 2 == 0 else acc_s[0]
                for j in range(2):
                    ti = 2 * gg + j
                    mm = t.matmul(buf[0:DH + 1, :],
                                  vha[:, ti, (DH + 1) * h:(DH + 1) * (h + 1)],
                                  probs[:, pslot(h, ti), :],
                                  start=(ti == 0), stop=(ti == NT - 1))
                if gg == 7:
                    mm.then_inc(sPE, 1)

            for h in range(H):
                qk(h, 0)
                for gg in range(1, 8):
                    qk(h, gg)
                    pv(h, gg - 1)
                pv(h, 7)
                if h == 0:
                    t.wait_ge(dL, 16 * 26)
                t.wait_ge(sDVE, dv_rr(h))
                t.matmul(rbc_ps[:, :], ones_r[:, :], rr_r[:, :],
                         start=True, stop=True).then_inc(sPE, 1)
            # P5
            t.wait_ge(dL, 16 * 25)
            t.wait_ge(sDVE, dv_aot(H - 1))
            for m in range(4):
                if m < 2:
                    t.wait_ge(sACT, ac_exp(H - 1, 6 + m))
                else:
                    t.wait_ge(sDVE, dv_p5(m - 2))
                for half in range(2):
                    for k in range(KT):
                        mm = t.matmul(acc_b[m % 2][:, 512 * half:512 * (half + 1)],
                                      aoT[:, k, P * m:P * (m + 1)],
                                      w_ar[:, k, 512 * half:512 * (half + 1)],
                                      start=(k == 0), stop=(k == KT - 1))
                mm.then_inc(sPE, 1)

        @block.scalar
        def _(a):
            for h in range(H):
                for gg in range(8):
                    a.wait_ge(sPE, pe_qk(h, gg))
                    if gg < 2:
                        if h >= 2:
                            a.wait_ge(sPOOL, po_nrm(h - 2))
                    elif h >= 1:
                        a.wait_ge(sDVE, dv_nrm(h - 1, (gg - 2) // 2))
                    s0 = pslot(h, 2 * gg)
                    a.activation(
                        probs[:, s0:s0 + 2, :].rearrange("p a b -> p (a b)"),
                        acc_b[gg % 2][:, :], Exp, scale=0.125).then_inc(sACT, 1)

        @block.vector
        def _(v):
            for st in range(NT):
                ones_view = vha[:, st, :].rearrange("p (h e) -> p h e", e=DH + 1)[:, :, DH:DH + 1]
                v.memset(ones_view, 1.0)
            for idx in range(32):
                skb, mt = idx // 8, idx % 8
                v.wait_ge(sPE, idx + 1)
                v.tensor_copy(khT[:, mt, SQ * skb:SQ * (skb + 1)],
                              acc_s[idx % 2][:, :]).then_inc(sDVE, 1)
            for mt in range(KT):
                v.wait_ge(sPE, pe_p2(mt))
                v.tensor_copy(qhT[:, mt, :], acc_s[(32 + mt) % 2][:, :]).then_inc(sDVE, 1)
            for st in range(NT):
                v.wait_ge(sPE, pe_p3(st))
                dst = vha[:, st, :].rearrange("p (h e) -> p h e", e=DH + 1)[:, :, 0:DH]
                v.tensor_copy(dst, acc_b[st % 2][:, :].rearrange("p (h d) -> p h d", d=DH)
                              ).then_inc(sDVE, 1)
            # P4
            for h in range(H):
                buf = pvo if h % 2 == 0 else acc_s[0]
                v.wait_ge(sPE, pe_pvo(h))
                v.reciprocal(rr[:, :], buf[DH:DH + 1, :])
                v.tensor_copy(rr_r[:, :], rr[:, :]).then_inc(sDVE, 1)
                v.wait_ge(sPE, pe_rbc(h))
                if h >= 1:
                    v.wait_ge(sPOOL, po_nrm(h - 1))     # rbc_sb WAR vs pool reads
                v.tensor_copy(rbc_sb[:, :], rbc_ps[:, :]).then_inc(sDVE, 1)
                off, mth = (h % 2) * DH, h // 2
                v.tensor_mul(aoT[off:off + DH, mth, :], buf[0:DH, :],
                             rbc_sb[0:DH, :]).then_inc(sDVE, 1)
                for c in range(3):
                    if c == 0 and h >= 1:
                        v.wait_ge(dS0, 16 * 3 * h)
                    s0 = pslot(h, 4 * c)
                    for t2 in range(4):
                        mm = v.tensor_mul(nrm_slot[c][:, t2, :],
                                          probs[:, s0 + t2, :], rbc_sb[:, :])
                    mm.then_inc(sDVE, 1)
            # P5 (osb aliases nrm slots 0,1)
            v.wait_ge(dS0, 16 * 48)
            for m in range(4):
                v.wait_ge(sPE, pe_p5(m))
                v.tensor_copy(osb[:, m, :], acc_b[m % 2][:, :]).then_inc(sDVE, 1)

        @block.sync
        def _(s):
            for h in range(H):
                for c in range(3):
                    s.wait_ge(sDVE, dv_nrm(h, c))
                    s.dma_start(attnT[h, 512 * c:512 * (c + 1), :]
                                .rearrange("(o p) f -> p o f", p=P),
                                nrm_slot[c][:, :, :].bitcast(f32)).then_inc(dS0, 16)
            s.wait_ge(sDVE, dv_p5(3))
            s.dma_start(out_p.rearrange("(o p) f -> p o f", p=P),
                        osb[:, :, :].bitcast(f32)).then_inc(dS0, 16)
            s.wait_ge(dS0, 16 * 49)
            s.wait_ge(dS1, 16 * 16)

    ctx.close()
    sems.close()
    return nc


def kernel(q, k, v, Wq, bq, Wk, bk, Wv, bv, Wo, bo):
    from concourse.bass_utils import run_bass_kernel_spmd

    for b_ in (bq, bk, bv, bo):
        assert np.max(np.abs(np.asarray(b_))) == 0.0, "nonzero biases unsupported"

    q, k, v = (np.asarray(x, dtype=np.float32) for x in (q, k, v))
    Wq, Wk, Wv, Wo = (np.asarray(x, dtype=np.float32) for x in (Wq, Wk, Wv, Wo))

    if "nc" not in _cache:
        _cache["nc"] = _build()
    nc = _cache["nc"]

    wqT, wkT, wvT, woT = (_round_f32r(W.T) for W in (Wq, Wk, Wv, Wo))
    kT = [_round_f32r(k[b].T) for b in range(B)]
    vT = [_round_f32r(v[b].T) for b in range(B)]
    qT = [_round_f32r(q[b].T) for b in range(B)]

    in_maps = []
    for c in range(NCORES):
        b, sl = c // SLABS, c % SLABS
        in_maps.append({
            "wqT": wqT, "wkT": wkT, "wvT": wvT, "woT": woT,
            "ones": np.ones((1, P), dtype=np.float32),
            "kT": kT[b], "vT": vT[b],
            "qT": np.ascontiguousarray(qT[b][:, SQ * sl:SQ * (sl + 1)]),
        })

    res = run_bass_kernel_spmd(nc, in_maps, list(range(NCORES)))
    _cache["last_result"] = res

    out = np.empty((B, S, DM), dtype=np.float32)
    attn = np.empty((B, H, S, S), dtype=np.float32)
    for c in range(NCORES):
        b, sl = c // SLABS, c % SLABS
        r = res.results[c]
        out[b, SQ * sl:SQ * (sl + 1), :] = r["out_p"]
        attn[b, :, SQ * sl:SQ * (sl + 1), :] = r["attnT"].transpose(0, 2, 1)
    return out, attn


# revision 21
# speedup vs baseline: 1.0829x; 1.0829x over previous
"""Multi-head attention (16 heads, d_model=1024, B=2, S=2048) on 8 TRN2 cores.

Sharding: core c -> batch b = c//4, query slab s = c%4 (512 rows).
Each core projects K/V for its whole batch (replicated over the 4 slab
cores of that batch), projects its Q slab, computes transposed scores
per head (scores_T[sk,sq] = K_h Q_h^T), exp without max-subtraction
(scores are O(5)), gets softmax denominators from a ones-column appended
to V during the PV matmul, normalizes, and runs the output projection
for its slab (no cross-core reduction needed). attn is produced
transposed per (b,h) and transposed back on host during unshard.

Raw Bass with manual semaphores: this walrus build allows only ONE
sync-wait per DMA/matmul instruction, so multi-deps ride on standalone
wait_ge instructions. The softmax reciprocal row is replicated across
partitions with a K=1 ones-matmul on the PE. Normalize multiplies are
split DVE (sk-tiles 0-11) / GpSimd (12-15); all stores issue from SP.
Wq is prefetched into a scratch arena that later holds probs/nrm
(disjoint lifetimes), so projection phases overlap their weight loads.

Matmul dtypes: projections + output projection in float32r (fp32 with
11-bit mantissa at full PE rate; inputs pre-rounded on host), attention
operands (K^T/Q^T heads, V+ones, probs) in bf16, accumulation fp32.
"""
import numpy as np

B, S, DM, H, DH = 2, 2048, 1024, 16, 64
NCORES, SLABS = 8, 4
SQ = S // SLABS      # 512
P = 128
KT = DM // P         # 8
NT = S // P          # 16 key tiles

_cache = {}


def _round_f32r(a):
    a = np.ascontiguousarray(a, dtype=np.float32)
    u = a.view(np.uint32).astype(np.uint64)
    u = (u + 0x7FF + ((u >> 12) & 1)) & 0xFFFFF000
    return u.astype(np.uint32).view(np.float32)


def _build():
    import concourse.bass as bass
    import concourse.mybir as mybir

    f32, f32r, bf16 = mybir.dt.float32, mybir.dt.float32r, mybir.dt.bfloat16
    Exp = mybir.ActivationFunctionType.Exp

    nc = bass.Bass(trn_type="TRN2", target_bir_lowering=False)

    wqT = nc.dram_tensor("wqT", [DM, DM], f32r, kind="ExternalInput")
    wkT = nc.dram_tensor("wkT", [DM, DM], f32r, kind="ExternalInput")
    wvT = nc.dram_tensor("wvT", [DM, DM], f32r, kind="ExternalInput")
    woT = nc.dram_tensor("woT", [DM, DM], f32r, kind="ExternalInput")
    kTd = nc.dram_tensor("kT", [DM, S], f32r, kind="ExternalInput")
    vTd = nc.dram_tensor("vT", [DM, S], f32r, kind="ExternalInput")
    qTd = nc.dram_tensor("qT", [DM, SQ], f32r, kind="ExternalInput")
    onesd = nc.dram_tensor("ones", [1, P], f32r, kind="ExternalInput")
    attnT = nc.dram_tensor("attnT", [H, S, SQ], f32, kind="ExternalOutput")
    out_p = nc.dram_tensor("out_p", [SQ, DM], f32, kind="ExternalOutput")

    from contextlib import ExitStack
    ctx = ExitStack()
    sb, ps = nc.sbuf_tensor, nc.psum_tensor

    w_ar = ctx.enter_context(sb("w_ar", [P, KT, DM], f32r))          # 32K: wk, wv, wo
    in_ar = ctx.enter_context(sb("in_ar", [P, KT, SQ], f32r))        # 16K: kst/qst/vst
    khT = ctx.enter_context(sb("khT", [P, KT, S], bf16))             # 32K
    qhT = ctx.enter_context(sb("qhT", [P, KT, SQ], bf16))            # 8K
    vha = ctx.enter_context(sb("vha", [P, NT, H * (DH + 1)], bf16))  # 32.5K
    aoT = ctx.enter_context(sb("aoT", [P, KT, SQ], f32r))            # 16K
    scr = ctx.enter_context(sb("scr", [P, 8192], f32r))              # 32K: wq | nrm | osb
    probs = ctx.enter_context(sb("probs", [P, NT + 4, SQ], bf16))    # 20K ring of 20 tiles
    rbc_sb = ctx.enter_context(sb("rbc_sb", [P, SQ], f32))           # 2K
    rr = ctx.enter_context(sb("rr", [1, SQ], f32))
    rr_r = ctx.enter_context(sb("rr_r", [1, SQ], f32r))
    ones_r = ctx.enter_context(sb("ones_r", [1, P], f32r))

    acc_s = [ctx.enter_context(ps(f"acc_s{i}", [P, 512], f32)) for i in range(2)]
    acc_b = [ctx.enter_context(ps(f"acc_b{i}", [P, 1024], f32)) for i in range(2)]
    pvo = ctx.enter_context(ps("pvo", [P, SQ], f32))   # pvo buffers: [pvo, acc_s[0]]
    rbc_ps = ctx.enter_context(ps("rbc_ps", [P, SQ], f32))

    kst = [in_ar[:, :, :], in_ar[:, :, :]]          # single buffer
    vst = [in_ar[:, :, 128 * i:128 * (i + 1)] for i in range(2)]
    qst = aoT[:, :, :]       # aoT is idle until P4; qT loads here eagerly
    # scratch views (wq lifetime: P1..P2; nrm slots: P4; osb: P5) — all f32r
    wq = scr[:, :].rearrange("p (o f) -> p o f", f=DM)                      # [P,8,1024]
    nrm_slot = [scr[:, 2048 * i:2048 * (i + 1)]
                .rearrange("p (t f) -> p t f", f=SQ) for i in range(4)]     # [P,4,512] f32r
    osb = scr[:, 0:4096].rearrange("p (m f) -> p m f", f=DM)                # [P,4,1024]

    sems = ExitStack()
    dL = sems.enter_context(nc.semaphore("dL"))
    dS0 = sems.enter_context(nc.semaphore("dS0"))
    dS1 = sems.enter_context(nc.semaphore("dS1"))
    sPE = sems.enter_context(nc.semaphore("sPE"))
    sDVE = sems.enter_context(nc.semaphore("sDVE"))
    sACT = sems.enter_context(nc.semaphore("sACT"))
    sPOOL = sems.enter_context(nc.semaphore("sPOOL"))

    # milestones
    pe_p1 = lambda skb, mt: 8 * skb + mt + 1            # 1..32
    pe_p2 = lambda mt: 32 + mt + 1                      # 33..40
    pe_p3 = lambda st: 40 + st + 1                      # 41..56
    pe_qk = lambda h, g: 56 + 10 * h + g + 1
    pe_pvo = lambda h: 56 + 10 * h + 9
    pe_rbc = lambda h: 56 + 10 * h + 10                 # end P4: 216
    pe_p5 = lambda m: 216 + m + 1

    dv_rr = lambda h: 56 + 6 * h + 1
    dv_rbc = lambda h: 56 + 6 * h + 2
    dv_aot = lambda h: 56 + 6 * h + 3
    dv_nrm = lambda h, c: 56 + 6 * h + 4 + c            # c=0..2
    dv_p5 = lambda m: 152 + m + 1                       # 153..156

    ac_exp = lambda h, g: 8 * h + g + 1                 # 1..128
    po_nrm = lambda h: h + 1                            # 1..16
    pslot = lambda h, t: (16 * h + t) % 20              # probs ring slot

    # load order: 1 wk, 2 kst0, 3 kst1, 4 wq, 5 kst2, 6 kst3, 7 qst, 8 wv,
    #             9..24 vst, 25 wo
    with nc.Block() as block:

        @block.gpsimd
        def _(g):
            g.dma_start(w_ar[:], wkT.rearrange("(o p) f -> p o f", p=P)).then_inc(dL, 16)
            g.dma_start(kst[0], kTd[:, 0:SQ].rearrange("(o p) f -> p o f", p=P)).then_inc(dL, 16)
            g.dma_start(wq, wqT.rearrange("(o p) f -> p o f", p=P)).then_inc(dL, 16)
            g.wait_ge(sPE, 8)
            g.dma_start(kst[1], kTd[:, SQ:2 * SQ].rearrange("(o p) f -> p o f", p=P)).then_inc(dL, 16)
            g.dma_start(qst, qTd.rearrange("(o p) f -> p o f", p=P)).then_inc(dL, 16)
            for skb in (2, 3):
                g.wait_ge(sPE, 8 * skb)
                g.dma_start(kst[skb % 2], kTd[:, SQ * skb:SQ * (skb + 1)]
                            .rearrange("(o p) f -> p o f", p=P)).then_inc(dL, 16)
            g.wait_ge(sPE, 32)
            g.dma_start(w_ar[:], wvT.rearrange("(o p) f -> p o f", p=P)).then_inc(dL, 16)
            for st in range(NT):
                if st < 2:
                    g.wait_ge(sPE, 32)
                else:
                    g.wait_ge(sPE, pe_p3(st - 2))
                g.dma_start(vst[st % 2], vTd[:, P * st:P * (st + 1)]
                            .rearrange("(o p) f -> p o f", p=P)).then_inc(dL, 16)
            g.wait_ge(sPE, 56)
            g.dma_start(w_ar[:], woT.rearrange("(o p) f -> p o f", p=P)).then_inc(dL, 16)
            g.dma_start(ones_r[:, :], onesd[:, :]).then_inc(dL, 16)
            # P4: normalize sk-tiles 12..15 + own store stream
            for h in range(H):
                g.wait_ge(sDVE, dv_rbc(h))
                if h >= 1:
                    g.wait_ge(dS1, 16 * h)
                s0 = pslot(h, 12)
                for t2 in range(4):
                    mm = g.tensor_mul(nrm_slot[3][:, t2, :], probs[:, s0 + t2, :],
                                      rbc_sb[:, :])
                mm.then_inc(sPOOL, 1)

        @block.tensor
        def _(t):
            # P1: khT
            dl_need = {0: 2, 1: 4, 2: 6, 3: 7}
            for skb in range(4):
                t.wait_ge(dL, 16 * dl_need[skb])
                for mt in range(KT):
                    idx = 8 * skb + mt
                    if idx >= 2:
                        t.wait_ge(sDVE, idx - 1)
                    for k in range(KT):
                        mm = t.matmul(acc_s[idx % 2][:, :],
                                      w_ar[:, k, P * mt:P * (mt + 1)],
                                      kst[skb % 2][:, k, :],
                                      start=(k == 0), stop=(k == KT - 1))
                    mm.then_inc(sPE, 1)
            # P2: qhT (weights from scr alias, activations from aoT alias)
            t.wait_ge(dL, 16 * 5)
            for mt in range(KT):
                idx = 32 + mt
                t.wait_ge(sDVE, idx - 1)
                for k in range(KT):
                    mm = t.matmul(acc_s[idx % 2][:, :],
                                  wq[:, k, P * mt:P * (mt + 1)],
                                  qst[:, k, :],
                                  start=(k == 0), stop=(k == KT - 1))
                mm.then_inc(sPE, 1)
            # P3: vh
            for st in range(NT):
                t.wait_ge(dL, 16 * (9 + st))
                if st >= 2:
                    t.wait_ge(sDVE, 41 + (st - 2))
                for half in range(2):
                    for k in range(KT):
                        mm = t.matmul(acc_b[st % 2][:, 512 * half:512 * (half + 1)],
                                      vst[st % 2][:, k, :],
                                      w_ar[:, k, 512 * half:512 * (half + 1)],
                                      start=(k == 0), stop=(k == KT - 1))
                mm.then_inc(sPE, 1)
            # P4
            t.wait_ge(sDVE, 56)

            def qk(h, gg):
                off, mth = (h % 2) * DH, h // 2
                if h == 0 and gg < 2:
                    t.wait_ge(sDVE, 55 + gg)
                else:
                    t.wait_ge(sACT, ac_exp(h, gg) - 2)
                for j in range(2):
                    ti = 2 * gg + j
                    mm = t.matmul(acc_b[gg % 2][:, 512 * j:512 * (j + 1)],
                                  khT[off:off + DH, mth, P * ti:P * (ti + 1)],
                                  qhT[off:off + DH, mth, :],
                                  start=True, stop=True)
                mm.then_inc(sPE, 1)

            def pv(h, gg):
                t.wait_ge(sACT, ac_exp(h, gg))
                if gg == 0 and h >= 2:
                    t.wait_ge(sDVE, dv_aot(h - 2))
                buf = pvo if h % 2 == 0 else acc_s[0]
                for j in range(2):
                    ti = 2 * gg + j
                    mm = t.matmul(buf[0:DH + 1, :],
                                  vha[:, ti, (DH + 1) * h:(DH + 1) * (h + 1)],
                                  probs[:, pslot(h, ti), :],
                                  start=(ti == 0), stop=(ti == NT - 1))
                if gg == 7:
                    mm.then_inc(sPE, 1)

            for h in range(H):
                qk(h, 0)
                for gg in range(1, 8):
                    qk(h, gg)
                    pv(h, gg - 1)
                pv(h, 7)
                if h == 0:
                    t.wait_ge(dL, 16 * 26)
                t.wait_ge(sDVE, dv_rr(h))
                t.matmul(rbc_ps[:, :], ones_r[:, :], rr_r[:, :],
                         start=True, stop=True).then_inc(sPE, 1)
            # P5
            t.wait_ge(dL, 16 * 25)
            t.wait_ge(sDVE, dv_aot(H - 1))
            for m in range(4):
                if m < 2:
                    t.wait_ge(sACT, ac_exp(H - 1, 6 + m))
                else:
                    t.wait_ge(sDVE, dv_p5(m - 2))
                for half in range(2):
                    for k in range(KT):
                        mm = t.matmul(acc_b[m % 2][:, 512 * half:512 * (half + 1)],
                                      aoT[:, k, P * m:P * (m + 1)],
                                      w_ar[:, k, 512 * half:512 * (half + 1)],
                                      start=(k == 0), stop=(k == KT - 1))
                mm.then_inc(sPE, 1)

        @block.scalar
        def _(a):
            for h in range(H):
                for gg in range(8):
                    a.wait_ge(sPE, pe_qk(h, gg))
                    if gg < 2:
                        if h >= 2:
                            a.wait_ge(sPOOL, po_nrm(h - 2))
                    elif h >= 1:
                        a.wait_ge(sDVE, dv_nrm(h - 1, (gg - 2) // 2))
                    s0 = pslot(h, 2 * gg)
                    a.activation(
                        probs[:, s0:s0 + 2, :].rearrange("p a b -> p (a b)"),
                        acc_b[gg % 2][:, :], Exp, scale=0.125).then_inc(sACT, 1)

        @block.vector
        def _(v):
            for st in range(NT):
                ones_view = vha[:, st, :].rearrange("p (h e) -> p h e", e=DH + 1)[:, :, DH:DH + 1]
                v.memset(ones_view, 1.0)
            for idx in range(32):
                skb, mt = idx // 8, idx % 8
                v.wait_ge(sPE, idx + 1)
                v.tensor_copy(khT[:, mt, SQ * skb:SQ * (skb + 1)],
                              acc_s[idx % 2][:, :]).then_inc(sDVE, 1)
            for mt in range(KT):
                v.wait_ge(sPE, pe_p2(mt))
                v.tensor_copy(qhT[:, mt, :], acc_s[(32 + mt) % 2][:, :]).then_inc(sDVE, 1)
            for st in range(NT):
                v.wait_ge(sPE, pe_p3(st))
                dst = vha[:, st, :].rearrange("p (h e) -> p h e", e=DH + 1)[:, :, 0:DH]
                v.tensor_copy(dst, acc_b[st % 2][:, :].rearrange("p (h d) -> p h d", d=DH)
                              ).then_inc(sDVE, 1)
            # P4
            for h in range(H):
                buf = pvo if h % 2 == 0 else acc_s[0]
                v.wait_ge(sPE, pe_pvo(h))
                v.reciprocal(rr[:, :], buf[DH:DH + 1, :])
                v.tensor_copy(rr_r[:, :], rr[:, :]).then_inc(sDVE, 1)
                v.wait_ge(sPE, pe_rbc(h))
                if h >= 1:
                    v.wait_ge(sPOOL, po_nrm(h - 1))     # rbc_sb WAR vs pool reads
                v.tensor_copy(rbc_sb[:, :], rbc_ps[:, :]).then_inc(sDVE, 1)
                off, mth = (h % 2) * DH, h // 2
                v.tensor_mul(aoT[off:off + DH, mth, :], buf[0:DH, :],
                             rbc_sb[0:DH, :]).then_inc(sDVE, 1)
                for c in range(3):
                    if c == 0 and h >= 1:
                        v.wait_ge(dS0, 16 * 3 * h)
                    s0 = pslot(h, 4 * c)
                    for t2 in range(4):
                        mm = v.tensor_mul(nrm_slot[c][:, t2, :],
                                          probs[:, s0 + t2, :], rbc_sb[:, :])
                    mm.then_inc(sDVE, 1)
            # P5 (osb aliases nrm slots 0,1)
            v.wait_ge(dS0, 16 * 48)
            for m in range(4):
                v.wait_ge(sPE, pe_p5(m))
                v.tensor_copy(osb[:, m, :], acc_b[m % 2][:, :]).then_inc(sDVE, 1)

        @block.sync
        def _(s):
            for h in range(H):
                for c in range(3):
                    s.wait_ge(sDVE, dv_nrm(h, c))
                    s.dma_start(attnT[h, 512 * c:512 * (c + 1), :]
                                .rearrange("(o p) f -> p o f", p=P),
                                nrm_slot[c][:, :, :].bitcast(f32)).then_inc(dS0, 16)
                s.wait_ge(sPOOL, po_nrm(h))
                s.dma_start(attnT[h, 512 * 3:512 * 4, :]
                            .rearrange("(o p) f -> p o f", p=P),
                            nrm_slot[3][:, :, :].bitcast(f32)).then_inc(dS1, 16)
            s.wait_ge(sDVE, dv_p5(3))
            s.dma_start(out_p.rearrange("(o p) f -> p o f", p=P),
                        osb[:, :, :].bitcast(f32)).then_inc(dS0, 16)
            s.wait_ge(dS0, 16 * 49)
            s.wait_ge(dS1, 16 * 16)

    ctx.close()
    sems.close()
    return nc


def kernel(q, k, v, Wq, bq, Wk, bk, Wv, bv, Wo, bo):
    from concourse.bass_utils import run_bass_kernel_spmd

    for b_ in (bq, bk, bv, bo):
        assert np.max(np.abs(np.asarray(b_))) == 0.0, "nonzero biases unsupported"

    q, k, v = (np.asarray(x, dtype=np.float32) for x in (q, k, v))
    Wq, Wk, Wv, Wo = (np.asarray(x, dtype=np.float32) for x in (Wq, Wk, Wv, Wo))

    if "nc" not in _cache:
        _cache["nc"] = _build()
    nc = _cache["nc"]

    wqT, wkT, wvT, woT = (_round_f32r(W.T) for W in (Wq, Wk, Wv, Wo))
    kT = [_round_f32r(k[b].T) for b in range(B)]
    vT = [_round_f32r(v[b].T) for b in range(B)]
    qT = [_round_f32r(q[b].T) for b in range(B)]

    in_maps = []
    for c in range(NCORES):
        b, sl = c // SLABS, c % SLABS
        in_maps.append({
            "wqT": wqT, "wkT": wkT, "wvT": wvT, "woT": woT,
            "ones": np.ones((1, P), dtype=np.float32),
            "kT": kT[b], "vT": vT[b],
            "qT": np.ascontiguousarray(qT[b][:, SQ * sl:SQ * (sl + 1)]),
        })

    res = run_bass_kernel_spmd(nc, in_maps, list(range(NCORES)))
    _cache["last_result"] = res

    out = np.empty((B, S, DM), dtype=np.float32)
    attn = np.empty((B, H, S, S), dtype=np.float32)
    for c in range(NCORES):
        b, sl = c // SLABS, c % SLABS
        r = res.results[c]
        out[b, SQ * sl:SQ * (sl + 1), :] = r["out_p"]
        attn[b, :, SQ * sl:SQ * (sl + 1), :] = r["attnT"].transpose(0, 2, 1)
    return out, attn
